# revision 3
# baseline (speedup 1.0000x reference)
"""Multi-head causal attention (B=2, T=2048, D=1024, H=16, HD=64) on 8 TRN2
NeuronCores.

Sharding: batch x head-group. Core c handles batch c//4 and heads
[4*(c%4), 4*(c%4)+4). Wq/Wk/Wv are split column-wise, Wo row-wise; each core
produces a full [T, D] partial output (its 4 heads' contribution, after
per-head softmax normalization and its Wo row-block), which the host sums
across the 4 cores of each batch and adds the bias to.

Per-core kernel layout (all matmuls contract along SBUF partitions):
  xT [D, T] f32r (host passes x[b].T), weights f32r.
  QT/KT computed transposed [2*64hd, T] per head-pair (lhsT = w, rhs = xT).
  V computed natural [T, 4*64hd] (lhsT = xT, rhs = wv), stored bf16 with a
  ones-column appended per head (stride 66) so the P@V matmul also produces
  the softmax row-sums (M = 65).
  Scores are computed transposed, ST[k, q] (lhsT = KT, rhs = QT), exact-causal
  (q >= 128*kt per k-tile), exp'd on ACT (scale=1/8 fused) to bf16 ET tiles;
  the strictly-lower triangle of the leading 128x128 diagonal block is zeroed
  with a multiplicative mask. CT' = V'.T @ ET accumulates [65, 512] per
  q-chunk in PSUM; partition 64 is the softmax denominator.
  Reciprocal row-sums are broadcast across partitions via a DRAM bounce and
  multiplied into CT (DVE, writes packed f32r CT_g [128c, T] tiles), then
  out[t, do] = CT_g.T @ wo (fp16 partial) is DMA'd out.
"""

import math

import numpy as np

T, D = 2048, 1024
NH, HD = 16, 64
HPC = 4  # heads per core
NCORES = 8
ND = D // 128  # 8 d-tiles
NT = T // 128  # 16 t/k-tiles
NQ = T // 512  # 4 q-chunks

_NC = None


def _build_nc():
    import concourse.mybir as mybir
    import concourse.tile as tile
    from concourse import bacc
    from concourse.masks import make_upper_triangular

    f32 = mybir.dt.float32
    f32r = mybir.dt.float32r
    bf16 = mybir.dt.bfloat16
    fp16 = mybir.dt.float16

    nc = bacc.Bacc("TRN2", target_bir_lowering=False, debug=False, num_devices=NCORES)

    xT_d = nc.dram_tensor("xT", [D, T], f32, kind="ExternalInput").ap()
    wq_d = nc.dram_tensor("wq", [D, HPC * HD], f32, kind="ExternalInput").ap()
    wk_d = nc.dram_tensor("wk", [D, HPC * HD], f32, kind="ExternalInput").ap()
    wv_d = nc.dram_tensor("wv", [D, HPC * HD], f32, kind="ExternalInput").ap()
    wo_d = nc.dram_tensor("wo", [HPC * HD, D], f32, kind="ExternalInput").ap()
    out_d = nc.dram_tensor("out", [T, D], fp16, kind="ExternalOutput").ap()
    rscr = nc.dram_tensor("rscr", [16, 512], f32).ap()
    rscr2 = nc.dram_tensor("rscr2", [16, 512], f32).ap()

    Exp = mybir.ActivationFunctionType.Exp

    with tile.TileContext(nc) as tc:
        with (
            tc.tile_pool(name="const", bufs=1) as constp,
            tc.tile_pool(name="qk", bufs=1) as qkp,
            tc.tile_pool(name="vpool", bufs=1) as vp,
            tc.tile_pool(name="wop", bufs=1) as wop,
        ):
            mask = constp.tile([128, 128], bf16, name="mask")
            make_upper_triangular(nc, mask[:], val=1.0, diag=True)

            # persistent tiles
            QT = [qkp.tile([128, T], f32r, name=f"QT{g}") for g in range(2)]
            KT = [qkp.tile([128, T], f32r, name=f"KT{g}") for g in range(2)]
            vsb = [vp.tile([128, 66 * HPC], bf16, name=f"v{tt}") for tt in range(NT)]
            wo_sb = [wop.tile([128, D], f32r, name=f"wo{gi}") for gi in range(2)]

            # ---------------- Phase A: load + project ----------------
            with (
                tc.tile_pool(name="xtr", bufs=1) as xtp,
                tc.tile_pool(name="astage", bufs=4) as stp,
                tc.tile_pool(name="wtiles", bufs=1) as wtp,
                tc.tile_pool(name="psA", bufs=2, space="PSUM") as psA,
                tc.tile_pool(name="psV", bufs=2, space="PSUM") as psV,
            ):
                def staged_load(dst, src, wd=512):
                    # DMA fp32 chunks into small staging tiles, cast to f32r dst
                    n = dst.shape[1]
                    for c0 in range(0, n, wd):
                        cw = min(wd, n - c0)
                        st = stp.tile([128, wd], f32, name=f"stage_{dst.name}_{c0}", tag="stage")
                        nc.sync.dma_start(st[:, 0:cw], src[:, c0 : c0 + cw])
                        nc.vector.tensor_copy(dst[:, c0 : c0 + cw], st[:, 0:cw])

                xtr = []
                for dt in range(ND):
                    xr = xtp.tile([128, T], f32r, name=f"xtr{dt}", tag=f"xtr{dt}")
                    staged_load(xr, xT_d[128 * dt : 128 * (dt + 1), :])
                    xtr.append(xr)

                wtiles = {}
                for wname, wd in [("wq", wq_d), ("wk", wk_d), ("wv", wv_d)]:
                    tiles = []
                    for dt in range(ND):
                        wr = wtp.tile([128, 256], f32r, name=f"{wname}r{dt}", tag=f"{wname}r{dt}")
                        staged_load(wr, wd[128 * dt : 128 * (dt + 1), :])
                        tiles.append(wr)
                    wtiles[wname] = tiles
                for gi in range(2):
                    staged_load(wo_sb[gi], wo_d[128 * gi : 128 * (gi + 1), :])

                # QT/KT: [128(2 heads), T] per pair-group
                for wname, outs in [("wq", QT), ("wk", KT)]:
                    for g in range(2):
                        for c in range(NQ):
                            ps = psA.tile([128, 512], f32, name=f"pj_{wname}{g}_{c}", tag="pj")
                            for dt in range(ND):
                                nc.tensor.matmul(
                                    ps[:],
                                    wtiles[wname][dt][:, 128 * g : 128 * (g + 1)],
                                    xtr[dt][:, 512 * c : 512 * (c + 1)],
                                    start=(dt == 0),
                                    stop=(dt == ND - 1),
                                )
                            nc.vector.tensor_copy(outs[g][:, 512 * c : 512 * (c + 1)], ps[:])

                # V natural [T, 4*HD] -> V' bf16 [128, 264] per t-tile
                for tt in range(NT):
                    ps = psV.tile([128, 256], f32, name=f"vps{tt}", tag="vps")
                    for dt in range(ND):
                        nc.tensor.matmul(
                            ps[:],
                            xtr[dt][:, 128 * tt : 128 * (tt + 1)],
                            wtiles["wv"][dt][:],
                            start=(dt == 0),
                            stop=(dt == ND - 1),
                        )
                    nc.any.memset(vsb[tt][:, 64 : 66 * HPC : 66], 1.0)
                    for h in range(HPC):
                        nc.vector.tensor_copy(
                            vsb[tt][:, 66 * h : 66 * h + 64], ps[:, 64 * h : 64 * (h + 1)]
                        )

            # ---------------- Phase B: attention per head ----------------
            import contextlib
            _stg_ctx = contextlib.ExitStack()
            stgp = _stg_ctx.enter_context(tc.tile_pool(name="stg", bufs=1))
            with (
                tc.tile_pool(name="et", bufs=2) as etp,
                tc.tile_pool(name="psST", bufs=2, space="PSUM") as psST,
                tc.tile_pool(name="psCT", bufs=2, space="PSUM") as psCT,
            ):
                stg = {}
                for h in range(HPC):
                    g, half = h // 2, h % 2
                    p0 = 64 * half
                    ets = []
                    for kt in range(NT):
                        w = T - 128 * kt
                        et = etp.tile([128, w], bf16, name=f"et_h{h}_kt{kt}", tag=f"et{kt}")
                        ets.append(et)
                        nsub = (w + 1023) // 1024
                        for sub in range(nsub):
                            sw = min(1024, w - 1024 * sub)
                            q0 = 128 * kt + 1024 * sub
                            ps = psST.tile([128, sw], f32, name=f"st_h{h}_k{kt}_s{sub}", tag="st")
                            for c in range((sw + 511) // 512):
                                n = min(512, sw - 512 * c)
                                nc.tensor.matmul(
                                    ps[:, 512 * c : 512 * c + n],
                                    KT[g][p0 : p0 + 64, 128 * kt : 128 * (kt + 1)],
                                    QT[g][p0 : p0 + 64, q0 + 512 * c : q0 + 512 * c + n],
                                    start=True,
                                    stop=True,
                                )
                            nc.scalar.activation(
                                et[:, 1024 * sub : 1024 * sub + sw],
                                ps[:, 0:sw],
                                Exp,
                                scale=0.125,
                            )
                        # zero strictly-lower triangle of the diagonal block
                        nc.gpsimd.tensor_mul(et[:, 0:128], et[:, 0:128], mask[:])

                    for j in range(NQ):
                        ct = psCT.tile([65, 512], f32, name=f"ct_h{h}_j{j}", tag="ct")
                        nkt = 4 * j + 4
                        for kt in range(nkt):
                            etoff = 512 * j - 128 * kt
                            if etoff >= 0:
                                n, psoff, ecol = 512, 0, etoff
                            else:
                                n, psoff, ecol = 512 + etoff, -etoff, 0
                            nc.tensor.matmul(
                                ct[0:65, psoff : psoff + n],
                                vsb[kt][:, 66 * h : 66 * h + 65],
                                ets[kt][:, ecol : ecol + n],
                                start=(kt == 0),
                                stop=(kt == nkt - 1),
                            )
                        s = stgp.tile([65, 512], f32r, name=f"stg_h{h}_j{j}")
                        stg[(h, j)] = s
                        nc.vector.tensor_copy(s[:], ct[:])
                        nc.sync.dma_start(
                            rscr[4 * h + j : 4 * h + j + 1, :], s[64:65, :].bitcast(f32)
                        )

            # ---------------- normalize + output projection ----------------
            with (
                tc.tile_pool(name="ctg", bufs=1) as ctgp,
                tc.tile_pool(name="norm", bufs=1) as normp,
                tc.tile_pool(name="rb", bufs=4) as rbp,
                tc.tile_pool(name="oh", bufs=3) as ohp,
                tc.tile_pool(name="psO", bufs=2, space="PSUM") as psO,
            ):
                CTG = [ctgp.tile([128, T], f32r, name=f"ctg{gi}") for gi in range(2)]
                rs_all = normp.tile([16, 512], f32, name="rs_all")
                nc.sync.dma_start(rs_all[:], rscr[:])
                recip = normp.tile([16, 512], f32, name="recip")
                nc.vector.reciprocal(recip[:], rs_all[:])
                nc.sync.dma_start(rscr2[:], recip[:])

                for h in range(HPC):
                    g, half = h // 2, h % 2
                    for j in range(NQ):
                        idx = 4 * h + j
                        rb = rbp.tile([64, 512], f32, name=f"rb{idx}", tag="rb")
                        nc.sync.dma_start(rb[:], rscr2[idx : idx + 1, :].partition_broadcast(64))
                        nc.vector.tensor_mul(
                            CTG[g][64 * half : 64 * half + 64, 512 * j : 512 * (j + 1)],
                            stg[(h, j)][0:64, :].bitcast(f32),
                            rb[:],
                        )

                for tt in range(NT):
                    ps = psO.tile([128, D], f32, name=f"ops{tt}", tag="ops")
                    for gi in range(2):
                        for dc in range(2):
                            nc.tensor.matmul(
                                ps[:, 512 * dc : 512 * (dc + 1)],
                                CTG[gi][:, 128 * tt : 128 * (tt + 1)],
                                wo_sb[gi][:, 512 * dc : 512 * (dc + 1)],
                                start=(gi == 0),
                                stop=(gi == 1),
                            )
                    oh = ohp.tile([128, D], fp16, name=f"oh{tt}", tag="oh")
                    nc.vector.tensor_copy(oh[:], ps[:])
                    nc.sync.dma_start(out_d[128 * tt : 128 * (tt + 1), :], oh[:])

            _stg_ctx.close()

    nc.compile()
    return nc


def _get_nc():
    global _NC
    if _NC is None:
        _NC = _build_nc()
    return _NC


def kernel(x, wq, wk, wv, wo, bo):
    from concourse.bass_utils import run_bass_kernel_spmd

    x = np.asarray(x, dtype=np.float32)
    wq = np.asarray(wq, dtype=np.float32)
    wk = np.asarray(wk, dtype=np.float32)
    wv = np.asarray(wv, dtype=np.float32)
    wo = np.asarray(wo, dtype=np.float32)
    bo = np.asarray(bo, dtype=np.float32)

    nc = _get_nc()
    in_maps = []
    for c in range(NCORES):
        b, g4 = c // 4, c % 4
        cs = slice(256 * g4, 256 * (g4 + 1))
        in_maps.append(
            {
                "xT": np.ascontiguousarray(x[b].T),
                "wq": np.ascontiguousarray(wq[:, cs]),
                "wk": np.ascontiguousarray(wk[:, cs]),
                "wv": np.ascontiguousarray(wv[:, cs]),
                "wo": np.ascontiguousarray(wo[cs, :]),
            }
        )
    res = run_bass_kernel_spmd(nc, in_maps, core_ids=list(range(NCORES))).results
    out = np.zeros((2, T, D), dtype=np.float32)
    for c in range(NCORES):
        out[c // 4] += res[c]["out"].astype(np.float32)
    out += bo[None, None, :]
    return out
